# revision 11
# baseline (speedup 1.0000x reference)
"""Trainium2 Bass kernel for nn_BranchingGNN (bipartite GNN message passing).

Strategy (destination-sharded, edge-parallel gather):
  - Partition the 20000 patterns into 160 groups (<=128 each) and the 50000
    items into 392 groups, balancing per-group edge counts (LPT). Each of the
    8 cores owns 20 pattern groups and 49 item groups. Node tables live in
    DRAM in permuted (core, group, slot) row order.
  - Phase 1 computes h_item (replicated, all rows) and the core's own h_pat
    rows with PE matmuls.
  - Each message-passing direction: bulk `dma_gather` of the source rows for
    all edges of a destination group, segment-sum via one-hot matmuls
    accumulated in PSUM ([H x 128slot] += gathered[e,H]^T @ onehot[e,slot]),
    then the MLP + relu + residual update of the core's own rows, then an
    AllGather so every core has the full updated table for the next gather.
  - int16 gather indices cap tables at 32768 rows, so gathers from the item
    table are split into lo (< 32768) / hi halves with separate edge lists.

All index preprocessing / padding / permutation happens on the host in numpy
(integer work only); every FLOP on feature data runs on the NeuronCores.
"""

import numpy as np

N, M, E = 50000, 20000, 1000000
DI, DP, H = 64, 64, 128
ROUNDS = 2
C = 8                  # cores
NBI = 49               # item groups per core  (8*49*128 = 50176 >= N)
NBP = 20               # pattern groups per core (8*20*128 = 20480 >= M)
NGI, NGP = C * NBI, C * NBP
RI, RP = NGI * 128, NGP * 128      # padded table rows: 50176, 20480
SPLIT = 32768
P = 128

TRACE = False          # test harness sets kernel.TRACE = True for profiling
LAST_RESULT = {}       # exec_time etc. stashed here when TRACE

_cache = {}


# ----------------------------------------------------------------------------
# host-side integer preprocessing
# ----------------------------------------------------------------------------

def _lpt_groups(deg, ngroups, cap=P):
    """Assign nodes to ngroups groups of <=cap, balancing sum(deg) (LPT)."""
    import heapq
    order = np.argsort(-deg, kind="stable")
    heap = [(0, g) for g in range(ngroups)]
    heapq.heapify(heap)
    count = np.zeros(ngroups, np.int64)
    load = np.zeros(ngroups, np.int64)
    gid = np.empty(deg.shape[0], np.int64)
    slot = np.empty(deg.shape[0], np.int64)
    for v in order:
        while True:
            l, g = heapq.heappop(heap)
            if count[g] < cap:
                break
        gid[v] = g
        slot[v] = count[g]
        count[g] += 1
        load[g] += deg[v]
        if count[g] < cap:
            heapq.heappush(heap, (load[g], g))
    return gid, slot, load


def _groups_to_cores(load_g, per_core):
    """Assign groups to C cores (per_core each), balancing total load."""
    import heapq
    order = np.argsort(-load_g, kind="stable")
    heap = [(0, c) for c in range(C)]
    heapq.heapify(heap)
    nblk = np.zeros(C, np.int64)
    core_of = np.empty(load_g.shape[0], np.int64)
    block_of = np.empty(load_g.shape[0], np.int64)
    for g in order:
        while True:
            l, c = heapq.heappop(heap)
            if nblk[c] < per_core:
                break
        core_of[g] = c
        block_of[g] = nblk[c]
        nblk[c] += 1
        heapq.heappush(heap, (l + load_g[g], c))
    return core_of, block_of


def _build_edge_arrays(dst_row, src_row, nblocks):
    """Group edges by destination (core, block); pad each block's list to a
    uniform T tiles of 128. Returns (T, idx [C, L] int32, slot [C, L] f32)
    with pad entries idx=0 / slot=-1."""
    core = dst_row // (nblocks * P)
    block = (dst_row // P) % nblocks
    slot = dst_row % P
    key = core * nblocks + block
    ngroups = C * nblocks
    counts = np.bincount(key, minlength=ngroups)
    T = int(np.ceil(counts.max() / P))
    L = nblocks * T * P
    order = np.argsort(key, kind="stable")
    starts = np.zeros(ngroups + 1, np.int64)
    np.cumsum(counts, out=starts[1:])
    pos = np.arange(key.shape[0]) - starts[key[order]]
    idx = np.zeros((C, L), np.int64)
    slo = np.full((C, L), -1.0, np.float32)
    ko = key[order]
    co = ko // nblocks
    bo = ko % nblocks
    j = bo * (T * P) + pos
    flat = co * L + j
    idx.reshape(-1)[flat] = src_row[order]
    slo.reshape(-1)[flat] = slot[order]
    return T, idx, slo


def _pack_idx(arr):
    """[C, L] -> [C, 128, L//16] int16: idx j at partition 16q + j%16 (all 8
    gpsimd stripes replicated), column j//16."""
    Cn, L = arr.shape
    a = arr.reshape(Cn, L // 16, 16).transpose(0, 2, 1)      # [C,16,L/16]
    return np.tile(a, (1, 8, 1)).astype(np.int16)


def _pack_slot(arr):
    """[C, L] -> [C, 128, L//128] f32: tile t, partition j%128."""
    Cn, L = arr.shape
    return np.ascontiguousarray(
        arr.reshape(Cn, L // P, P).transpose(0, 2, 1)).astype(np.float32)


def _preprocess(i_idx, p_idx):
    deg_i = np.bincount(i_idx, minlength=N)
    gid_i, slot_i, load_i = _lpt_groups(deg_i, NGI)
    core_i, block_i = _groups_to_cores(load_i, NBI)
    perm_item = core_i[gid_i] * (NBI * P) + block_i[gid_i] * P + slot_i

    deg_p = np.bincount(p_idx, minlength=M)
    gid_p, slot_p, load_p = _lpt_groups(deg_p, NGP)
    core_p, block_p = _groups_to_cores(load_p, NBP)
    perm_pat = core_p[gid_p] * (NBP * P) + block_p[gid_p] * P + slot_p

    src_i = perm_item[i_idx]          # item table row per edge
    dst_p = perm_pat[p_idx]           # pattern table row per edge

    lo = src_i < SPLIT
    T_LO, idx1lo, slot1lo = _build_edge_arrays(dst_p[lo], src_i[lo], NBP)
    T_HI, idx1hi, slot1hi = _build_edge_arrays(
        dst_p[~lo], src_i[~lo] - SPLIT, NBP)
    T2, idx2, slot2 = _build_edge_arrays(src_i, dst_p, NBI)

    return dict(
        perm_item=perm_item, perm_pat=perm_pat,
        T_LO=T_LO, T_HI=T_HI, T2=T2,
        idx1lo=_pack_idx(idx1lo), slot1lo=_pack_slot(slot1lo),
        idx1hi=_pack_idx(idx1hi), slot1hi=_pack_slot(slot1hi),
        idx2=_pack_idx(idx2), slot2=_pack_slot(slot2),
    )


# ----------------------------------------------------------------------------
# device program
# ----------------------------------------------------------------------------

def _build_program(T_LO, T_HI, T2):
    import concourse.bacc as bacc
    import concourse.mybir as mybir
    import concourse.tile as tile
    from concourse.masks import make_identity

    F32 = mybir.dt.float32
    I16 = mybir.dt.int16
    RELU = mybir.ActivationFunctionType.Relu
    EQ = mybir.AluOpType.is_equal
    ADD = mybir.AluOpType.add

    nc = bacc.Bacc("TRN2", num_devices=C)

    item_xT = nc.dram_tensor("item_xT", [DI, RI], F32, kind="ExternalInput")
    item_xT_own = nc.dram_tensor("item_xT_own", [DI, NBI * P], F32, kind="ExternalInput")
    pat_xT = nc.dram_tensor("pat_xT", [DP, NBP * P], F32, kind="ExternalInput")
    w_item_in = nc.dram_tensor("w_item", [DI, H], F32, kind="ExternalInput")
    w_pat_in = nc.dram_tensor("w_pat", [DP, H], F32, kind="ExternalInput")
    w_i2p_in = nc.dram_tensor("w_i2p", [H, H], F32, kind="ExternalInput")
    w_p2i_in = nc.dram_tensor("w_p2i", [H, H], F32, kind="ExternalInput")
    b_item_in = nc.dram_tensor("b_item", [P, H], F32, kind="ExternalInput")
    b_pat_in = nc.dram_tensor("b_pat", [P, H], F32, kind="ExternalInput")
    b_i2p_in = nc.dram_tensor("b_i2p", [H, 1], F32, kind="ExternalInput")
    b_p2i_in = nc.dram_tensor("b_p2i", [H, 1], F32, kind="ExternalInput")
    idx1lo_in = nc.dram_tensor("idx1lo", [P, NBP * T_LO * 8], I16, kind="ExternalInput")
    idx1hi_in = nc.dram_tensor("idx1hi", [P, NBP * T_HI * 8], I16, kind="ExternalInput")
    idx2_in = nc.dram_tensor("idx2", [P, NBI * T2 * 8], I16, kind="ExternalInput")
    slot1lo_in = nc.dram_tensor("slot1lo", [P, NBP * T_LO], F32, kind="ExternalInput")
    slot1hi_in = nc.dram_tensor("slot1hi", [P, NBP * T_HI], F32, kind="ExternalInput")
    slot2_in = nc.dram_tensor("slot2", [P, NBI * T2], F32, kind="ExternalInput")

    out_item = nc.dram_tensor("out_item", [NBI * P, H], F32, kind="ExternalOutput")
    out_pat = nc.dram_tensor("out_pat", [NBP * P, H], F32, kind="ExternalOutput")

    h_item_t0 = nc.dram_tensor("h_item_t0", [RI, H], F32)
    h_item_t1 = nc.dram_tensor("h_item_t1", [RI, H], F32)
    h_pat_t = [nc.dram_tensor(f"h_pat_t{r}", [RP, H], F32)
               for r in range(ROUNDS)]
    my_pat = [nc.dram_tensor(f"my_pat{r}", [NBP * P, H], F32)
              for r in range(ROUNDS)]
    my_item = nc.dram_tensor("my_item", [NBI * P, H], F32)

    groups = [list(range(C))]
    XCH = 8                      # phase-1 item chunk: 8 blocks of 128 rows
    NCHUNK = NGI // XCH          # 49 chunks

    with tile.TileContext(nc, num_cores=C) as tc:
        with (
            tc.tile_pool(name="const", bufs=1) as cpool,
            tc.tile_pool(name="work", bufs=4) as wpool,
            tc.tile_pool(name="gath", bufs=4) as gpool,
            tc.tile_pool(name="xc", bufs=2) as xpool,
            tc.tile_pool(name="stage", bufs=2) as stpool,
            tc.tile_pool(name="psum", bufs=2, space="PSUM") as ppool,
        ):
            ident = cpool.tile([P, P], F32)
            make_identity(nc, ident[:])
            iota = cpool.tile([P, P], F32)
            nc.gpsimd.iota(iota[:], pattern=[[1, P]], base=0,
                           channel_multiplier=0,
                           allow_small_or_imprecise_dtypes=True)
            w_item = cpool.tile([DI, H], F32)
            nc.sync.dma_start(out=w_item[:], in_=w_item_in[:])
            w_pat = cpool.tile([DP, H], F32)
            nc.sync.dma_start(out=w_pat[:], in_=w_pat_in[:])
            w_i2p = cpool.tile([H, H], F32)
            nc.sync.dma_start(out=w_i2p[:], in_=w_i2p_in[:])
            w_p2i = cpool.tile([H, H], F32)
            nc.sync.dma_start(out=w_p2i[:], in_=w_p2i_in[:])
            b_item = cpool.tile([P, H], F32)
            nc.sync.dma_start(out=b_item[:], in_=b_item_in[:])
            b_pat = cpool.tile([P, H], F32)
            nc.sync.dma_start(out=b_pat[:], in_=b_pat_in[:])
            b_i2p = cpool.tile([H, 1], F32)
            nc.sync.dma_start(out=b_i2p[:], in_=b_i2p_in[:])
            b_p2i = cpool.tile([H, 1], F32)
            nc.sync.dma_start(out=b_p2i[:], in_=b_p2i_in[:])
            idx1lo = cpool.tile([P, NBP * T_LO * 8], I16)
            nc.sync.dma_start(out=idx1lo[:], in_=idx1lo_in[:])
            idx1hi = cpool.tile([P, NBP * T_HI * 8], I16)
            nc.sync.dma_start(out=idx1hi[:], in_=idx1hi_in[:])
            idx2 = cpool.tile([P, NBI * T2 * 8], I16)
            nc.sync.dma_start(out=idx2[:], in_=idx2_in[:])
            slot1lo = cpool.tile([P, NBP * T_LO], F32)
            nc.sync.dma_start(out=slot1lo[:], in_=slot1lo_in[:])
            slot1hi = cpool.tile([P, NBP * T_HI], F32)
            nc.sync.dma_start(out=slot1hi[:], in_=slot1hi_in[:])
            slot2 = cpool.tile([P, NBI * T2], F32)
            nc.sync.dma_start(out=slot2[:], in_=slot2_in[:])

            own_item = cpool.tile([P, NBI * P], F32)
            own_pat = cpool.tile([P, NBP * P], F32)

            # ---------------- phase 1: h_item (all rows, replicated) -------
            for ch in range(NCHUNK):
                xc = xpool.tile([DI, XCH * P], F32, tag="xc")
                nc.sync.dma_start(
                    out=xc[:], in_=item_xT[:, ch * XCH * P:(ch + 1) * XCH * P])
                st = stpool.tile([P, XCH * P], F32, tag="st")
                for k in range(XCH):
                    gb = ch * XCH + k          # global block id
                    mm = ppool.tile([P, H], F32, tag="mm")
                    nc.tensor.matmul(mm[:], lhsT=xc[:, k * P:(k + 1) * P],
                                     rhs=w_item[:], start=True, stop=True)
                    nc.vector.tensor_tensor(
                        out=st[:, k * P:(k + 1) * P], in0=mm[:], in1=b_item[:],
                        op=ADD)
                    nc.scalar.activation(
                        out=st[:, k * P:(k + 1) * P],
                        in_=st[:, k * P:(k + 1) * P], func=RELU)
                nc.sync.dma_start(
                    out=h_item_t0[ch * XCH * P:(ch + 1) * XCH * P, :]
                    .rearrange("(b s) h -> s b h", s=P),
                    in_=st[:].rearrange("p (b h) -> p b h", h=H))
            # ---------------- phase 1: own h_item rows (per-core input) ----
            for k in range(NBI):
                oxk = xpool.tile([DI, P], F32, tag="oxk")
                nc.sync.dma_start(out=oxk[:],
                                  in_=item_xT_own[:, k * P:(k + 1) * P])
                mm = ppool.tile([P, H], F32, tag="mm")
                nc.tensor.matmul(mm[:], lhsT=oxk[:], rhs=w_item[:],
                                 start=True, stop=True)
                nc.vector.tensor_tensor(
                    out=own_item[:, k * P:(k + 1) * P], in0=mm[:],
                    in1=b_item[:], op=ADD)
                nc.scalar.activation(
                    out=own_item[:, k * P:(k + 1) * P],
                    in_=own_item[:, k * P:(k + 1) * P], func=RELU)

            # ---------------- phase 1: own h_pat rows ----------------------
            for k in range(NBP):
                oxk = xpool.tile([DP, P], F32, tag="oxk")
                nc.sync.dma_start(out=oxk[:], in_=pat_xT[:, k * P:(k + 1) * P])
                mm = ppool.tile([P, H], F32, tag="mm")
                nc.tensor.matmul(mm[:], lhsT=oxk[:], rhs=w_pat[:],
                                 start=True, stop=True)
                nc.vector.tensor_tensor(
                    out=own_pat[:, k * P:(k + 1) * P], in0=mm[:], in1=b_pat[:],
                    op=ADD)
                nc.scalar.activation(
                    out=own_pat[:, k * P:(k + 1) * P],
                    in_=own_pat[:, k * P:(k + 1) * P], func=RELU)

            # ---------------- message-passing rounds -----------------------
            for r in range(ROUNDS):
                item_src = h_item_t0 if r == 0 else h_item_t1

                # --- item -> pattern ---
                GCH = 8          # gather chunk (tiles): <=1024 rows/instr
                for pb in range(NBP):
                    acc = ppool.tile([H, P], F32, tag="acc")
                    # (half, table_ap, idx_tile, slot_tile, T, first, last)
                    halves = [
                        (item_src[0:SPLIT, :], idx1lo, slot1lo, T_LO, True, False),
                        (item_src[SPLIT:RI, :], idx1hi, slot1hi, T_HI, False, True),
                    ]
                    for tab, idxt, slott, T, first, last in halves:
                        for t0 in range(0, T, GCH):
                            tw = min(GCH, T - t0)
                            g = gpool.tile([P, tw, H], F32, tag="g")
                            c0 = (pb * T + t0) * 8
                            nc.gpsimd.dma_gather(
                                g[:], tab, idxt[:, c0:c0 + tw * 8],
                                tw * P, tw * P, H)
                            for tt in range(tw):
                                t = t0 + tt
                                oh = wpool.tile([P, P], F32, tag="oh")
                                nc.vector.tensor_tensor(
                                    out=oh[:],
                                    in0=slott[:, pb * T + t:pb * T + t + 1]
                                    .to_broadcast([P, P]),
                                    in1=iota[:], op=EQ)
                                nc.tensor.matmul(
                                    acc[:], lhsT=g[:, tt, :], rhs=oh[:],
                                    start=(first and t == 0),
                                    stop=(last and t == T - 1))
                    msgT = wpool.tile([H, P], F32, tag="msgT")
                    nc.vector.tensor_copy(out=msgT[:], in_=acc[:])
                    mm2 = ppool.tile([H, P], F32, tag="mm2")
                    nc.tensor.matmul(mm2[:], lhsT=w_i2p[:], rhs=msgT[:],
                                     start=True, stop=True)
                    m2 = wpool.tile([H, P], F32, tag="m2")
                    nc.scalar.activation(out=m2[:], in_=mm2[:], func=RELU,
                                         bias=b_i2p[:, :1], scale=1.0)
                    tr = ppool.tile([P, H], F32, tag="tr")
                    nc.tensor.transpose(out=tr[:], in_=m2[:], identity=ident[:])
                    sl = slice(pb * P, (pb + 1) * P)
                    nc.vector.tensor_tensor(out=own_pat[:, sl],
                                            in0=own_pat[:, sl], in1=tr[:],
                                            op=ADD)
                    nc.scalar.activation(out=own_pat[:, sl],
                                         in_=own_pat[:, sl], func=RELU)
                nc.sync.dma_start(
                    out=my_pat[r][:].rearrange("(b s) h -> s b h", s=P),
                    in_=own_pat[:].rearrange("p (b h) -> p b h", h=H))
                if r == ROUNDS - 1:
                    nc.sync.dma_start(
                        out=out_pat[:].rearrange("(b s) h -> s b h", s=P),
                        in_=own_pat[:].rearrange("p (b h) -> p b h", h=H))
                nc.gpsimd.collective_compute(
                    "AllGather", mybir.AluOpType.bypass,
                    replica_groups=groups,
                    ins=[my_pat[r][:].opt()], outs=[h_pat_t[r][:].opt()])

                # --- pattern -> item ---
                for ib in range(NBI):
                    acc = ppool.tile([H, P], F32, tag="acc")
                    for t0 in range(0, T2, GCH):
                        tw = min(GCH, T2 - t0)
                        g2 = gpool.tile([P, tw, H], F32, tag="g")
                        c0 = (ib * T2 + t0) * 8
                        nc.gpsimd.dma_gather(
                            g2[:], h_pat_t[r][:], idx2[:, c0:c0 + tw * 8],
                            tw * P, tw * P, H)
                        for tt in range(tw):
                            t = t0 + tt
                            oh = wpool.tile([P, P], F32, tag="oh")
                            nc.vector.tensor_tensor(
                                out=oh[:],
                                in0=slot2[:, ib * T2 + t:ib * T2 + t + 1]
                                .to_broadcast([P, P]),
                                in1=iota[:], op=EQ)
                            nc.tensor.matmul(
                                acc[:], lhsT=g2[:, tt, :], rhs=oh[:],
                                start=(t == 0), stop=(t == T2 - 1))
                    msgT = wpool.tile([H, P], F32, tag="msgT")
                    nc.vector.tensor_copy(out=msgT[:], in_=acc[:])
                    mm2 = ppool.tile([H, P], F32, tag="mm2")
                    nc.tensor.matmul(mm2[:], lhsT=w_p2i[:], rhs=msgT[:],
                                     start=True, stop=True)
                    m2 = wpool.tile([H, P], F32, tag="m2")
                    nc.scalar.activation(out=m2[:], in_=mm2[:], func=RELU,
                                         bias=b_p2i[:, :1], scale=1.0)
                    tr = ppool.tile([P, H], F32, tag="tr")
                    nc.tensor.transpose(out=tr[:], in_=m2[:], identity=ident[:])
                    sl = slice(ib * P, (ib + 1) * P)
                    nc.vector.tensor_tensor(out=own_item[:, sl],
                                            in0=own_item[:, sl], in1=tr[:],
                                            op=ADD)
                    nc.scalar.activation(out=own_item[:, sl],
                                         in_=own_item[:, sl], func=RELU)
                if r == 0:
                    nc.sync.dma_start(
                        out=my_item[:].rearrange("(b s) h -> s b h", s=P),
                        in_=own_item[:].rearrange("p (b h) -> p b h", h=H))
                    nc.gpsimd.collective_compute(
                        "AllGather", mybir.AluOpType.bypass,
                        replica_groups=groups,
                        ins=[my_item[:].opt()], outs=[h_item_t1[:].opt()])
                else:
                    nc.sync.dma_start(
                        out=out_item[:].rearrange("(b s) h -> s b h", s=P),
                        in_=own_item[:].rearrange("p (b h) -> p b h", h=H))

    nc.compile()
    return nc


# ----------------------------------------------------------------------------
# entry point
# ----------------------------------------------------------------------------

def kernel(item_feat, pattern_feat, i_idx, p_idx,
           W_item, b_item, W_pat, b_pat,
           W_i2p, b_i2p, W_p2i, b_p2i):
    from concourse.bass_utils import run_bass_kernel_spmd

    item_feat = np.asarray(item_feat, np.float32)
    pattern_feat = np.asarray(pattern_feat, np.float32)
    i_idx = np.asarray(i_idx).astype(np.int64)
    p_idx = np.asarray(p_idx).astype(np.int64)
    W_item = np.asarray(W_item, np.float32)
    b_item = np.asarray(b_item, np.float32)
    W_pat = np.asarray(W_pat, np.float32)
    b_pat = np.asarray(b_pat, np.float32)
    W_i2p = np.asarray(W_i2p, np.float32)
    b_i2p = np.asarray(b_i2p, np.float32)
    W_p2i = np.asarray(W_p2i, np.float32)
    b_p2i = np.asarray(b_p2i, np.float32)

    key = hash((i_idx.tobytes(), p_idx.tobytes()))
    if key in _cache:
        pp, nc = _cache[key]
    else:
        pp = _preprocess(i_idx, p_idx)
        nc = _build_program(pp["T_LO"], pp["T_HI"], pp["T2"])
        _cache[key] = (pp, nc)

    perm_item, perm_pat = pp["perm_item"], pp["perm_pat"]

    item_xT = np.zeros((DI, RI), np.float32)
    item_xT[:, perm_item] = item_feat.T
    pat_xT = np.zeros((DP, RP), np.float32)
    pat_xT[:, perm_pat] = pattern_feat.T

    b_item_bc = np.ascontiguousarray(np.broadcast_to(b_item, (P, H)))
    b_pat_bc = np.ascontiguousarray(np.broadcast_to(b_pat, (P, H)))

    in_maps = []
    for c in range(C):
        in_maps.append({
            "item_xT": item_xT,
            "item_xT_own": np.ascontiguousarray(
                item_xT[:, c * NBI * P:(c + 1) * NBI * P]),
            "pat_xT": np.ascontiguousarray(
                pat_xT[:, c * NBP * P:(c + 1) * NBP * P]),
            "w_item": W_item, "w_pat": W_pat,
            "w_i2p": W_i2p, "w_p2i": W_p2i,
            "b_item": b_item_bc, "b_pat": b_pat_bc,
            "b_i2p": b_i2p.reshape(H, 1), "b_p2i": b_p2i.reshape(H, 1),
            "idx1lo": pp["idx1lo"][c], "idx1hi": pp["idx1hi"][c],
            "idx2": pp["idx2"][c],
            "slot1lo": pp["slot1lo"][c], "slot1hi": pp["slot1hi"][c],
            "slot2": pp["slot2"][c],
        })

    res = run_bass_kernel_spmd(nc, in_maps, list(range(C)), trace=TRACE)
    if TRACE:
        LAST_RESULT["exec_time_ns"] = res.exec_time_ns
        LAST_RESULT["mean_exec_time_ns"] = res.mean_exec_time_ns
        LAST_RESULT["profile_json"] = res.profile_json
        LAST_RESULT["instructions_and_trace"] = res.instructions_and_trace

    item_tab = np.concatenate([res.results[c]["out_item"] for c in range(C)], 0)
    pat_tab = np.concatenate([res.results[c]["out_pat"] for c in range(C)], 0)
    h_item = item_tab[perm_item]
    h_pat = pat_tab[perm_pat]
    return h_item, h_pat
